# revision 13
# baseline (speedup 1.0000x reference)
"""Trainium2 Bass kernel for nn_Attention_28338194219036.

GQA attention block (QKV proj + QK-RMSNorm + RoPE + causal SDPA + out proj)
for x:[2,2048,2048], 16 q-heads / 4 kv-heads, head_dim 128.

Distribution over 8 NeuronCores: 2-way data parallel on batch x 4-way tensor
parallel on heads (core c: batch c//4, TP rank c%4 -> q-heads 4r..4r+3,
kv-head r). Per token chunk a 4-rank AllGather exchanges head-shards of
y^T; each rank then computes its 512 output channels of Wo for that chunk.

v3 design (vs v2):
- Attention AV matmul emits y in NATURAL [token, head-dim] layout
  (lhsT=p, rhs=v): the softmax denominator becomes an N=1 matmul reusing
  the already-loaded p weights (was an N=qlen ones-matmul per key block,
  ~70k PE cycles/core), and the 1/l normalization becomes a per-partition
  tensor_scalar on the DVE (was a gpsimd partition_broadcast + DVE mul).
- y is transposed back to [head-dim, token] for the out-projection with
  the XBAR DMA-transpose (no PE cycles, no gpsimd).
- K's RMS-norm factor is folded into the exp() as a per-partition
  (per-key) activation scale; Q's factor is broadcast with a single
  f32r PE matmul (ones-column outer product) instead of a gpsimd
  partition_broadcast. The gpsimd/Pool queue now carries ONLY the
  AllGathers, so norm chains never queue behind a collective.
- Projection emission is two-pass (matmuls+row-sums first, RoPE/norm
  consumers second) so the in-order PE never waits on act/DVE latency.
- AllGather outputs live in Shared scratchpad; last chunk split into
  128-token slivers so the final AllGather hides behind out-proj work.
"""

import os
import sys

for _p in ("/opt/trn_rl_repo", "/root/.axon_site/_ro/trn_rl_repo"):
    if os.path.isdir(_p) and _p not in sys.path:
        sys.path.append(_p)

import numpy as np

B, T, C = 2, 2048, 2048
NH, NKV, HD = 16, 4, 128
TP = 4            # tensor-parallel group size
NCORES = 8
QH = NH // TP     # q-heads per core (4)
QD = QH * HD      # q channels per core (512)
TC = 4            # projection token chunks of 512
TCH = T // TC     # 512
CCH = C // 128    # 16 channel chunks
ROPE_BASE = 10000.0
SCALE = 1.0 / float(np.sqrt(HD))
EPS = float(np.finfo(np.float32).eps)
NEG = -1.0e9
REPEAT = 1
NO_COLLECTIVE = False

# attention sub-chunks: (qoff, qlen); tail split into 128-token slivers so the
# final AllGathers hide behind out-projection PE work
SUBS = [(0, 512), (512, 512), (1024, 512), (1536, 256), (1792, 128), (1920, 128)]

_CACHE = {}


def _build_nc():
    import concourse.mybir as mybir
    import concourse.tile as tile
    from concourse import bacc

    F32 = mybir.dt.float32
    F32R = mybir.dt.float32r
    BF16 = mybir.dt.bfloat16
    AF = mybir.ActivationFunctionType

    nc = bacc.Bacc("TRN2", target_bir_lowering=False, debug=False, num_devices=NCORES)

    x_in = nc.dram_tensor("x_in", [128, TC * CCH * TCH], BF16, kind="ExternalInput")
    wq_in = nc.dram_tensor("wq_in", [128, CCH * QD], BF16, kind="ExternalInput")
    wk_in = nc.dram_tensor("wk_in", [128, CCH * HD], BF16, kind="ExternalInput")
    wv_in = nc.dram_tensor("wv_in", [128, CCH * HD], BF16, kind="ExternalInput")
    wo_in = nc.dram_tensor("wo_in", [128, CCH * QD], BF16, kind="ExternalInput")
    cc_in = nc.dram_tensor("cc_in", [128, T], F32, kind="ExternalInput")
    ss_in = nc.dram_tensor("ss_in", [128, T], F32, kind="ExternalInput")
    mask_in = nc.dram_tensor("mask_in", [128, 128], F32, kind="ExternalInput")
    outT = nc.dram_tensor("outT", [QD, T], F32, kind="ExternalOutput")

    with tile.TileContext(nc) as tc:
        for _rep in range(REPEAT):
            with (
                tc.tile_pool(name="drp", bufs=1, space="DRAM") as drp,
                tc.tile_pool(name="pw", bufs=1) as pw,
                tc.tile_pool(name="px", bufs=1) as px,
                tc.tile_pool(name="pat", bufs=1) as pat,
                tc.tile_pool(name="psp", bufs=1, space="PSUM") as psp,
            ):
                y_loc = [drp.tile([QD, ql], BF16, name=f"y_loc{i}") for i, (_, ql) in enumerate(SUBS)]
                y_all = [drp.tile([C, ql], BF16, name=f"y_all{i}") for i, (_, ql) in enumerate(SUBS)]

                # ---- persistent SBUF state ----
                ones_b = pw.tile([128, 1], BF16, name="ones_b")
                nc.any.memset(ones_b[:], 1.0)
                ones_rf = pw.tile([1, 128], F32, name="ones_rf")
                nc.any.memset(ones_rf[:], 1.0)
                ones_r = pw.tile([1, 128], F32R, name="ones_r")
                nc.scalar.activation(ones_r[:], ones_rf[:], AF.Copy)
                epst = pw.tile([1, 1], F32, name="epst")
                nc.any.memset(epst[:], EPS)
                epstK = pw.tile([1, 1], F32, name="epstK")
                nc.any.memset(epstK[:], EPS * HD)

                wk_s = pw.tile([128, CCH * HD], BF16, name="wk_s")
                nc.sync.dma_start(wk_s[:, : CCH * HD // 2], wk_in[:, : CCH * HD // 2])
                wq_s = pw.tile([128, CCH * QD], BF16, name="wq_s")
                wv_s = pw.tile([128, CCH * HD], BF16, name="wv_s")
                wo_s = pw.tile([128, CCH * QD], BF16, name="wo_s")
                mask_tri = pw.tile([128, 128], F32, name="mask_tri")

                qhat = [pw.tile([128, T], BF16, name=f"qhat{h}") for h in range(QH)]
                khat = pw.tile([128, T], BF16, name="khat")
                vnat = pw.tile([128, T], BF16, name="vnat")
                # per-key-block exp scale: SCALE / rms_k, [128 keys, 16 blocks]
                srk_sb = pw.tile([128, CCH], F32, name="srk_sb")

                def load_x(tci):
                    """One x chunk -> SBUF [128, CCH*TCH] bf16, split in 4 DMAs
                    so the first projection matmuls start early."""
                    x_t = px.tile([128, CCH * TCH], BF16, tag="x", bufs=2, name=f"x{tci}")
                    base = tci * CCH * TCH
                    step = 4 * TCH
                    for i in range(4):
                        nc.sync.dma_start(
                            x_t[:, i * step : (i + 1) * step],
                            x_in[:, base + i * step : base + (i + 1) * step],
                        )
                    return x_t

                def load_tabs(tci):
                    tsl = slice(tci * TCH, (tci + 1) * TCH)
                    cc_t = px.tile([128, TCH], F32, tag="cc", bufs=2, name=f"cc{tci}")
                    nc.sync.dma_start(cc_t[:], cc_in[:, tsl])
                    ss_t = px.tile([128, TCH], F32, tag="ss", bufs=2, name=f"ss{tci}")
                    nc.sync.dma_start(ss_t[:], ss_in[:, tsl])
                    return cc_t, ss_t

                x_tiles = {0: load_x(0)}
                nc.sync.dma_start(wk_s[:, CCH * HD // 2 :], wk_in[:, CCH * HD // 2 :])
                tab_tiles = {0: load_tabs(0)}
                nc.sync.dma_start(wv_s[:], wv_in[:])
                nc.sync.dma_start(wq_s[:], wq_in[:])
                nc.sync.dma_start(mask_tri[:], mask_in[:])
                nc.sync.dma_start(wo_s[:], wo_in[:])

                def rms_sums(x_ps, is_k):
                    """Pass A of norm: square + row-sum + sqrt + reciprocal.
                    Returns (xs, rin): the f32 copy of the projection and the
                    [1, TCH] reciprocal-rms row (bf16 for K, f32 for Q)."""
                    sq = px.tile([128, TCH], BF16, tag="sq", bufs=2, name="sq")
                    nc.scalar.activation(sq[:], x_ps[:], AF.Square)
                    xs = px.tile([128, TCH], F32, tag="xs", bufs=3, name="xs")
                    nc.scalar.activation(xs[:], x_ps[:], AF.Copy)
                    msq = psp.tile([128, TCH], F32, tag="y", bufs=2, name="msq")
                    nc.tensor.matmul(msq[0:1, :], lhsT=ones_b[:], rhs=sq[:], start=True, stop=True)
                    srt = px.tile([1, TCH], F32, tag="srt", bufs=2, name="srt")
                    if is_k:
                        # srt = sqrt(sum(k^2) + HD*eps) = rms_k/SCALE
                        nc.scalar.activation(srt[:], msq[0:1, :], AF.Sqrt, bias=epstK[:], scale=1.0)
                        rin = px.tile([1, TCH], BF16, tag="rinK", bufs=2, name="rinK")
                        with nc.allow_low_precision(reason="per-key 1/rms in bf16: ±0.4% independent per key"):
                            nc.vector.reciprocal(rin[:], srt[:])
                    else:
                        nc.scalar.activation(srt[:], msq[0:1, :], AF.Sqrt, bias=epst[:], scale=1.0 / HD)
                        rin = px.tile([1, TCH], F32R, tag="rinQ", bufs=2, name="rinQ")
                        with nc.allow_low_precision(reason="1/rms in f32r (19-bit mantissa) for the broadcast matmul"):
                            nc.vector.reciprocal(rin[:], srt[:])
                    return xs, rin

                def rope_mix(xs, cc_t, ss_t):
                    """u = xs*cc + swap64(xs*ss_preswapped); returns u (f32)."""
                    t1 = px.tile([128, TCH], F32, tag="t1", bufs=2, name="t1")
                    nc.vector.tensor_mul(t1[:], xs[:], ss_t[:])
                    t2 = px.tile([128, TCH], F32, tag="t2", bufs=2, name="t2")
                    nc.sync.dma_start(t2[0:64, :], t1[64:128, :])
                    nc.sync.dma_start(t2[64:128, :], t1[0:64, :])
                    u = px.tile([128, TCH], F32, tag="u", bufs=2, name="u")
                    nc.vector.tensor_mul(u[:], xs[:], cc_t[:])
                    return u, t2

                def proj(tci):
                    # prefetch next chunk's x + tables first
                    if tci + 1 < TC:
                        x_tiles[tci + 1] = load_x(tci + 1)
                        tab_tiles[tci + 1] = load_tabs(tci + 1)
                    x_t = x_tiles.pop(tci)
                    cc_t, ss_t = tab_tiles.pop(tci)
                    tsl = slice(tci * TCH, (tci + 1) * TCH)

                    # ---- pass A: all PE projection matmuls + rms row-sums ----
                    k_ps = psp.tile([128, TCH], F32, tag="G", bufs=4, name="k_ps")
                    for cci in range(CCH):
                        nc.tensor.matmul(
                            k_ps[:], lhsT=wk_s[:, cci * HD : (cci + 1) * HD],
                            rhs=x_t[:, cci * TCH : (cci + 1) * TCH],
                            start=(cci == 0), stop=(cci == CCH - 1),
                        )
                    k_xs, k_rin = rms_sums(k_ps, is_k=True)

                    # V projection, directly in [token, hd] layout (x as lhsT)
                    v_ps = psp.tile([128, TCH], F32, tag="G", bufs=4, name="v_ps")
                    for jj in range(4):
                        for cci in range(CCH):
                            nc.tensor.matmul(
                                v_ps[:, jj * HD : (jj + 1) * HD],
                                lhsT=x_t[:, cci * TCH + jj * 128 : cci * TCH + (jj + 1) * 128],
                                rhs=wv_s[:, cci * HD : (cci + 1) * HD],
                                start=(cci == 0), stop=(cci == CCH - 1),
                            )
                    nc.any.tensor_copy(vnat[:, tsl], v_ps[:])

                    q_parts = []
                    for h in range(QH):
                        q_ps = psp.tile([128, TCH], F32, tag="G", bufs=4, name="q_ps")
                        for cci in range(CCH):
                            nc.tensor.matmul(
                                q_ps[:], lhsT=wq_s[:, cci * QD + h * HD : cci * QD + (h + 1) * HD],
                                rhs=x_t[:, cci * TCH : (cci + 1) * TCH],
                                start=(cci == 0), stop=(cci == CCH - 1),
                            )
                        q_parts.append(rms_sums(q_ps, is_k=False))

                    # ---- pass B: K-norm column-ify + RoPE chains ----
                    # K: srk column per key block via 4 tiny matmuls, then one copy
                    kct = psp.tile([128, 4], F32, tag="ln", bufs=2, name="kct")
                    for b in range(4):
                        nc.tensor.matmul(
                            kct[:, b : b + 1],
                            lhsT=k_rin[0:1, b * 128 : (b + 1) * 128],
                            rhs=ones_b[0:1, 0:1],
                            start=(b == 0), stop=(b == 3),
                        )
                    nc.scalar.activation(
                        srk_sb[:, tci * 4 : (tci + 1) * 4], kct[:, 0:4], AF.Copy
                    )
                    u, t2 = rope_mix(k_xs, cc_t, ss_t)
                    nc.vector.tensor_add(khat[:, tsl], u[:], t2[:])

                    # Q: broadcast 1/rms via f32r ones-column matmul, then normalize
                    for h in range(QH):
                        q_xs, q_rin = q_parts[h]
                        rbc = psp.tile([128, TCH], F32, tag="y", bufs=2, name="rbc")
                        nc.tensor.matmul(
                            rbc[:],
                            lhsT=ones_r[:],
                            rhs=q_rin[:],
                            start=True, stop=True,
                        )
                        u, t2 = rope_mix(q_xs, cc_t, ss_t)
                        nc.vector.tensor_add(u[:], u[:], t2[:])
                        nc.vector.tensor_mul(qhat[h][:, tsl], u[:], rbc[:])

                def attn(si):
                    qoff, qlen = SUBS[si]
                    kb_tot = (qoff + qlen) // 128
                    dstart = qoff // 128  # first diagonal key block
                    nsub = qlen // 128
                    LEAD = 3  # score blocks emitted ahead of their y/l pair
                    ps = {}

                    def s_exp(h, j):
                        off = max(0, (j - dstart) * 128)
                        s_ps = psp.tile([128, TCH], F32, tag="G", bufs=4, name="s_ps")
                        nc.tensor.matmul(
                            s_ps[:, off:qlen],
                            lhsT=khat[:, j * 128 : (j + 1) * 128],
                            rhs=qhat[h][:, qoff + off : qoff + qlen],
                            start=True, stop=True,
                        )
                        if j >= dstart:
                            nc.vector.tensor_add(
                                s_ps[:, off : off + 128],
                                s_ps[:, off : off + 128],
                                mask_tri[:],
                            )
                        p = pat.tile([128, TCH], BF16, tag="p", bufs=8, name="p")
                        nc.scalar.activation(
                            p[:, off:qlen], s_ps[:, off:qlen], AF.Exp,
                            scale=srk_sb[:, j : j + 1],
                        )
                        ps[(h, j)] = p

                    blocks = [(h, j) for h in range(QH) for j in range(kb_tot)]
                    for h, j in blocks[:LEAD]:
                        s_exp(h, j)
                    y_nat = l_nat = None
                    for idx, (h, j) in enumerate(blocks):
                        if idx + LEAD < len(blocks):
                            s_exp(*blocks[idx + LEAD])
                        if j == 0:
                            y_nat = psp.tile([128, TCH], F32, tag="y", bufs=2, name="y_nat")
                            l_nat = psp.tile([128, 4], F32, tag="ln", bufs=2, name="l_nat")
                        p = ps.pop((h, j))
                        # one lazy-zeroed accumulation group per PSUM tile: start
                        # only on the first write, stop only on the last; each
                        # b-block's first touch lands while its bytes are pending
                        for b in range(max(0, j - dstart), nsub):
                            nc.tensor.matmul(
                                y_nat[:, b * 128 : (b + 1) * 128],
                                lhsT=p[:, b * 128 : (b + 1) * 128],
                                rhs=vnat[:, j * 128 : (j + 1) * 128],
                                start=(j == 0 and b == 0), stop=(j == kb_tot - 1),
                            )
                            nc.tensor.matmul(
                                l_nat[:, b : b + 1],
                                lhsT=p[:, b * 128 : (b + 1) * 128],
                                rhs=ones_b[:, 0:1],
                                start=(j == 0 and b == 0), stop=(j == kb_tot - 1),
                            )
                        if j == kb_tot - 1:
                            rcl = pat.tile([128, 4], F32, tag="rcl", bufs=2, name="rcl")
                            nc.vector.reciprocal(rcl[:, :nsub], l_nat[:, :nsub])
                            yh_nat = pat.tile([128, TCH], BF16, tag="yhn", bufs=2, name="yh_nat")
                            for b in range(nsub):
                                nc.vector.tensor_scalar_mul(
                                    yh_nat[:, b * 128 : (b + 1) * 128],
                                    y_nat[:, b * 128 : (b + 1) * 128],
                                    rcl[:, b : b + 1],
                                )
                            yhT = pat.tile([128, TCH], BF16, tag="yhT", bufs=2, name="yhT")
                            nc.sync.dma_start(
                                yhT[:, :qlen].rearrange("p (b t) -> p b t", t=128),
                                yh_nat[:, :qlen],
                                transpose=True,
                            )
                            nc.sync.dma_start(
                                y_loc[si][h * 128 : (h + 1) * 128, :], yhT[:, :qlen]
                            )

                def allgather(si):
                    if NO_COLLECTIVE:
                        for q in range(TP):
                            nc.sync.dma_start(
                                y_all[si][q * QD : (q + 1) * QD, :], y_loc[si][:]
                            )
                    else:
                        import concourse.mybir as mybir

                        nc.gpsimd.collective_compute(
                            "AllGather",
                            mybir.AluOpType.bypass,
                            replica_groups=[[0, 1, 2, 3], [4, 5, 6, 7]],
                            ins=[y_loc[si][:]],
                            outs=[y_all[si][:]],
                        )

                yb_tiles = {}

                def oread(si):
                    # issue the gathered-y readback as soon as the AllGather is
                    # ordered, so it never queues behind later DMA traffic
                    qoff, qlen = SUBS[si]
                    yb = pat.tile([128, CCH * TCH], BF16, tag="yb", bufs=2, name="yb")
                    src = y_all[si][:].rearrange("(cci p) t -> p cci t", p=128)
                    dst = yb[:, : CCH * qlen].rearrange("p (cci t) -> p cci t", t=qlen)
                    nc.sync.dma_start(dst, src)
                    yb_tiles[si] = yb

                def outproj(si):
                    qoff, qlen = SUBS[si]
                    yb = yb_tiles.pop(si)
                    ob = pat.tile([128, 4 * TCH], F32, tag="ob", bufs=1, name="ob")
                    for jq in range(4):
                        o_ps = psp.tile([128, TCH], F32, tag="G", bufs=4, name="o_ps")
                        for cci in range(CCH):
                            nc.tensor.matmul(
                                o_ps[:, :qlen],
                                lhsT=wo_s[:, cci * QD + jq * 128 : cci * QD + (jq + 1) * 128],
                                rhs=yb[:, cci * qlen : (cci + 1) * qlen],
                                start=(cci == 0), stop=(cci == CCH - 1),
                            )
                        nc.scalar.activation(
                            ob[:, jq * qlen : (jq + 1) * qlen], o_ps[:, :qlen], AF.Copy
                        )
                    src = ob[:, : 4 * qlen].rearrange("p (jq t) -> p jq t", jq=4)
                    dst = outT[:, qoff : qoff + qlen].rearrange("(jq p) t -> p jq t", p=128)
                    nc.sync.dma_start(dst, src)

                # ---- emission schedule ----
                proj(0)
                proj(1)
                attn(0); allgather(0); oread(0)
                proj(2)
                attn(1); allgather(1); oread(1); outproj(0)
                proj(3)
                attn(2); allgather(2); oread(2); outproj(1)
                attn(3); allgather(3); oread(3); outproj(2)
                attn(4); allgather(4); oread(4); outproj(3)
                attn(5); allgather(5); oread(5); outproj(4)
                outproj(5)

    nc.compile()
    return nc


def _get_nc():
    if "nc" not in _CACHE:
        _CACHE["nc"] = _build_nc()
    return _CACHE["nc"]


def _lay(wT):
    """[C, M] (already transposed weight) -> [128, (C/128)*M] with channel
    blocks as column groups: out[p, cci*M + j] = wT[cci*128 + p, j]."""
    Cd, M = wT.shape
    return np.ascontiguousarray(
        wT.reshape(Cd // 128, 128, M).transpose(1, 0, 2).reshape(128, -1)
    )


def _host_constants():
    if "consts" in _CACHE:
        return _CACHE["consts"]
    inv_freq = 1.0 / (ROPE_BASE ** (np.arange(0, HD, 2, dtype=np.float64) / HD))
    freqs = np.outer(np.arange(T, dtype=np.float64), inv_freq)  # [T, 64]
    cos = np.cos(freqs).astype(np.float32).T  # [64, T]
    sin = np.sin(freqs).astype(np.float32).T
    ccT = np.ascontiguousarray(np.concatenate([cos, cos], axis=0))   # [128, T]
    # the kernel computes swap(x*ss) (swap applied AFTER the multiply), so the
    # sin table is pre-swapped: swap(x)*[+sin;-sin] == swap(x*[-sin;+sin])
    ssT = np.ascontiguousarray(np.concatenate([-sin, sin], axis=0))  # [128, T]
    ii = np.arange(128, dtype=np.int64)[:, None]
    cc = np.arange(128, dtype=np.int64)[None, :]
    masks = np.where(cc >= ii, 0.0, NEG).astype(np.float32)
    _CACHE["consts"] = (ccT, ssT, masks)
    return _CACHE["consts"]


def _in_maps(x, Wq, Wk, Wv, Wo):
    import ml_dtypes

    BF = ml_dtypes.bfloat16
    ccT, ssT, masks = _host_constants()
    maps = []
    for c in range(NCORES):
        b, r = divmod(c, TP)
        xT = x[b].T.astype(BF)  # [C, T]
        x_l = np.concatenate(
            [_lay(xT[:, t * TCH : (t + 1) * TCH]) for t in range(TC)], axis=1
        )
        maps.append(
            {
                "x_in": np.ascontiguousarray(x_l),
                "wq_in": _lay(Wq[r * QD : (r + 1) * QD, :].T.astype(BF)),
                "wk_in": _lay(Wk[r * HD : (r + 1) * HD, :].T.astype(BF)),
                "wv_in": _lay(Wv[r * HD : (r + 1) * HD, :].T.astype(BF)),
                "wo_in": _lay(Wo[r * QD : (r + 1) * QD, :].T.astype(BF)),
                "cc_in": ccT,
                "ss_in": ssT,
                "mask_in": masks,
            }
        )
    return maps


def _assemble(results):
    out = np.empty((B, T, C), dtype=np.float32)
    for c in range(NCORES):
        b, r = divmod(c, TP)
        out[b, :, r * QD : (r + 1) * QD] = results[c]["outT"].T
    return out


def kernel(x, Wq, Wk, Wv, Wo):
    from concourse.bass_utils import run_bass_kernel_spmd

    nc = _get_nc()
    maps = _in_maps(np.asarray(x), np.asarray(Wq), np.asarray(Wk), np.asarray(Wv), np.asarray(Wo))
    res = run_bass_kernel_spmd(nc, maps, list(range(NCORES)))
    return _assemble(res.results)


# revision 18
# speedup vs baseline: 1.1623x; 1.1623x over previous
"""Trainium2 Bass kernel for nn_Attention_28338194219036.

GQA attention block (QKV proj + QK-RMSNorm + RoPE + causal SDPA + out proj)
for x:[2,2048,2048], 16 q-heads / 4 kv-heads, head_dim 128.

Distribution over 8 NeuronCores: 2-way data parallel on batch x 4-way tensor
parallel on heads (core c: batch c//4, TP rank c%4 -> q-heads 4r..4r+3,
kv-head r). Per token chunk a 4-rank AllGather exchanges head-shards of
y^T; each rank then computes its 512 output channels of Wo for that chunk.

v3 design (vs v2):
- Attention AV matmul emits y in NATURAL [token, head-dim] layout
  (lhsT=p, rhs=v): the softmax denominator becomes an N=1 matmul reusing
  the already-loaded p weights (was an N=qlen ones-matmul per key block,
  ~70k PE cycles/core), and the 1/l normalization becomes a per-partition
  tensor_scalar on the DVE (was a gpsimd partition_broadcast + DVE mul).
- y is transposed back to [head-dim, token] for the out-projection with
  the XBAR DMA-transpose (no PE cycles, no gpsimd).
- K's RMS-norm factor is folded into the exp() as a per-partition
  (per-key) activation scale; Q's factor is broadcast with a single
  f32r PE matmul (ones-column outer product) instead of a gpsimd
  partition_broadcast. The gpsimd/Pool queue now carries ONLY the
  AllGathers, so norm chains never queue behind a collective.
- Projection emission is two-pass (matmuls+row-sums first, RoPE/norm
  consumers second) so the in-order PE never waits on act/DVE latency.
- AllGather outputs live in Shared scratchpad; last chunk split into
  128-token slivers so the final AllGather hides behind out-proj work.
"""

import os
import sys

for _p in ("/opt/trn_rl_repo", "/root/.axon_site/_ro/trn_rl_repo"):
    if os.path.isdir(_p) and _p not in sys.path:
        sys.path.append(_p)

import numpy as np

B, T, C = 2, 2048, 2048
NH, NKV, HD = 16, 4, 128
TP = 4            # tensor-parallel group size
NCORES = 8
QH = NH // TP     # q-heads per core (4)
QD = QH * HD      # q channels per core (512)
TC = 4            # projection token chunks of 512
TCH = T // TC     # 512
CCH = C // 128    # 16 channel chunks
ROPE_BASE = 10000.0
SCALE = 1.0 / float(np.sqrt(HD))
EPS = float(np.finfo(np.float32).eps)
NEG = -1.0e9
REPEAT = 1
NO_COLLECTIVE = False
TMODE = "mm"  # "xbar": XBAR DMA transpose; "mm": PE matmul vs identity

# attention sub-chunks: (qoff, qlen); tail split into 128-token slivers so the
# final AllGathers hide behind out-projection PE work
SUBS = [(0, 512), (512, 512), (1024, 512), (1536, 256), (1792, 128), (1920, 128)]

_CACHE = {}


def _build_nc():
    import concourse.mybir as mybir
    import concourse.tile as tile
    from concourse import bacc

    F32 = mybir.dt.float32
    F32R = mybir.dt.float32r
    BF16 = mybir.dt.bfloat16
    AF = mybir.ActivationFunctionType

    nc = bacc.Bacc("TRN2", target_bir_lowering=False, debug=False, num_devices=NCORES)

    x_in = nc.dram_tensor("x_in", [128, TC * CCH * TCH], BF16, kind="ExternalInput")
    wq_in = nc.dram_tensor("wq_in", [128, CCH * QD], BF16, kind="ExternalInput")
    wk_in = nc.dram_tensor("wk_in", [128, CCH * HD], BF16, kind="ExternalInput")
    wv_in = nc.dram_tensor("wv_in", [128, CCH * HD], BF16, kind="ExternalInput")
    wo_in = nc.dram_tensor("wo_in", [128, CCH * QD], BF16, kind="ExternalInput")
    cc_in = nc.dram_tensor("cc_in", [128, T], F32, kind="ExternalInput")
    ss_in = nc.dram_tensor("ss_in", [128, T], F32, kind="ExternalInput")
    mask_in = nc.dram_tensor("mask_in", [128, 128], F32, kind="ExternalInput")
    eye_in = nc.dram_tensor("eye_in", [128, 128], BF16, kind="ExternalInput")
    outT = nc.dram_tensor("outT", [QD, T], F32, kind="ExternalOutput")

    with tile.TileContext(nc) as tc:
        for _rep in range(REPEAT):
            with (
                tc.tile_pool(name="drp", bufs=1, space="DRAM") as drp,
                tc.tile_pool(name="pw", bufs=1) as pw,
                tc.tile_pool(name="px", bufs=1) as px,
                tc.tile_pool(name="pat", bufs=1) as pat,
                tc.tile_pool(name="psp", bufs=1, space="PSUM") as psp,
            ):
                y_loc = [drp.tile([QD, ql], BF16, name=f"y_loc{i}") for i, (_, ql) in enumerate(SUBS)]
                y_all = [drp.tile([C, ql], BF16, name=f"y_all{i}") for i, (_, ql) in enumerate(SUBS)]

                # ---- persistent SBUF state ----
                ones_b = pw.tile([128, 1], BF16, name="ones_b")
                nc.any.memset(ones_b[:], 1.0)
                ones_rf = pw.tile([1, 128], F32, name="ones_rf")
                nc.any.memset(ones_rf[:], 1.0)
                ones_r = pw.tile([1, 128], F32R, name="ones_r")
                nc.scalar.activation(ones_r[:], ones_rf[:], AF.Copy)
                epst = pw.tile([1, 1], F32, name="epst")
                nc.any.memset(epst[:], EPS)
                epstK = pw.tile([1, 1], F32, name="epstK")
                nc.any.memset(epstK[:], EPS * HD)

                wk_s = pw.tile([128, CCH * HD], BF16, name="wk_s")
                nc.sync.dma_start(wk_s[:, : CCH * HD // 2], wk_in[:, : CCH * HD // 2])
                wq_s = pw.tile([128, CCH * QD], BF16, name="wq_s")
                wv_s = pw.tile([128, CCH * HD], BF16, name="wv_s")
                wo_s = pw.tile([128, CCH * QD], BF16, name="wo_s")
                mask_tri = pw.tile([128, 128], F32, name="mask_tri")
                eye_s = pw.tile([128, 128], BF16, name="eye_s")
                if TMODE == "mm":
                    nc.sync.dma_start(eye_s[:], eye_in[:])

                qhat = [pw.tile([128, T], BF16, name=f"qhat{h}") for h in range(QH)]
                khat = pw.tile([128, T], BF16, name="khat")
                vnat = pw.tile([128, T], BF16, name="vnat")
                # per-key-block exp scale: SCALE / rms_k, [128 keys, 16 blocks]
                srk_sb = pw.tile([128, CCH], F32, name="srk_sb")

                def load_x(tci):
                    """One x chunk -> SBUF [128, CCH*TCH] bf16, split in 4 DMAs
                    so the first projection matmuls start early."""
                    x_t = px.tile([128, CCH * TCH], BF16, tag="x", bufs=2, name=f"x{tci}")
                    base = tci * CCH * TCH
                    step = 4 * TCH
                    for i in range(4):
                        nc.sync.dma_start(
                            x_t[:, i * step : (i + 1) * step],
                            x_in[:, base + i * step : base + (i + 1) * step],
                        )
                    return x_t

                def load_tabs(tci):
                    tsl = slice(tci * TCH, (tci + 1) * TCH)
                    cc_t = px.tile([128, TCH], F32, tag="cc", bufs=2, name=f"cc{tci}")
                    nc.sync.dma_start(cc_t[:], cc_in[:, tsl])
                    ss_t = px.tile([128, TCH], F32, tag="ss", bufs=2, name=f"ss{tci}")
                    nc.sync.dma_start(ss_t[:], ss_in[:, tsl])
                    return cc_t, ss_t

                x_tiles = {0: load_x(0)}
                nc.sync.dma_start(wk_s[:, CCH * HD // 2 :], wk_in[:, CCH * HD // 2 :])
                tab_tiles = {0: load_tabs(0)}
                nc.sync.dma_start(wv_s[:], wv_in[:])
                nc.sync.dma_start(wq_s[:], wq_in[:])
                nc.sync.dma_start(mask_tri[:], mask_in[:])
                nc.sync.dma_start(wo_s[:], wo_in[:])

                def rms_sums(x_ps, is_k):
                    """Pass A of norm: square + row-sum + sqrt + reciprocal.
                    Returns (xs, rin): the f32 copy of the projection and the
                    [1, TCH] reciprocal-rms row (bf16 for K, f32 for Q)."""
                    sq = px.tile([128, TCH], BF16, tag="sq", bufs=2, name="sq")
                    nc.scalar.activation(sq[:], x_ps[:], AF.Square)
                    xs = px.tile([128, TCH], F32, tag="xs", bufs=3, name="xs")
                    nc.scalar.activation(xs[:], x_ps[:], AF.Copy)
                    msq = psp.tile([128, TCH], F32, tag="y", bufs=2, name="msq")
                    nc.tensor.matmul(msq[0:1, :], lhsT=ones_b[:], rhs=sq[:], start=True, stop=True)
                    srt = px.tile([1, TCH], F32, tag="srt", bufs=2, name="srt")
                    if is_k:
                        # srt = sqrt(sum(k^2) + HD*eps) = rms_k/SCALE
                        nc.scalar.activation(srt[:], msq[0:1, :], AF.Sqrt, bias=epstK[:], scale=1.0)
                        rin = px.tile([1, TCH], BF16, tag="rinK", bufs=2, name="rinK")
                        with nc.allow_low_precision(reason="per-key 1/rms in bf16: ±0.4% independent per key"):
                            nc.vector.reciprocal(rin[:], srt[:])
                    else:
                        nc.scalar.activation(srt[:], msq[0:1, :], AF.Sqrt, bias=epst[:], scale=1.0 / HD)
                        rin = px.tile([1, TCH], F32R, tag="rinQ", bufs=2, name="rinQ")
                        with nc.allow_low_precision(reason="1/rms in f32r (19-bit mantissa) for the broadcast matmul"):
                            nc.vector.reciprocal(rin[:], srt[:])
                    return xs, rin

                def rope_mix(xs, cc_t, ss_t):
                    """u = xs*cc + swap64(xs*ss_preswapped); returns u (f32)."""
                    t1 = px.tile([128, TCH], F32, tag="t1", bufs=2, name="t1")
                    nc.vector.tensor_mul(t1[:], xs[:], ss_t[:])
                    t2 = px.tile([128, TCH], F32, tag="t2", bufs=2, name="t2")
                    nc.sync.dma_start(t2[0:64, :], t1[64:128, :])
                    nc.sync.dma_start(t2[64:128, :], t1[0:64, :])
                    u = px.tile([128, TCH], F32, tag="u", bufs=2, name="u")
                    nc.vector.tensor_mul(u[:], xs[:], cc_t[:])
                    return u, t2

                def proj(tci):
                    # prefetch next chunk's x + tables first
                    if tci + 1 < TC:
                        x_tiles[tci + 1] = load_x(tci + 1)
                        tab_tiles[tci + 1] = load_tabs(tci + 1)
                    x_t = x_tiles.pop(tci)
                    cc_t, ss_t = tab_tiles.pop(tci)
                    tsl = slice(tci * TCH, (tci + 1) * TCH)

                    # ---- pass A: all PE projection matmuls + rms row-sums ----
                    k_ps = psp.tile([128, TCH], F32, tag="G", bufs=4, name="k_ps")
                    for cci in range(CCH):
                        nc.tensor.matmul(
                            k_ps[:], lhsT=wk_s[:, cci * HD : (cci + 1) * HD],
                            rhs=x_t[:, cci * TCH : (cci + 1) * TCH],
                            start=(cci == 0), stop=(cci == CCH - 1),
                        )
                    k_xs, k_rin = rms_sums(k_ps, is_k=True)

                    # V projection, directly in [token, hd] layout (x as lhsT)
                    v_ps = psp.tile([128, TCH], F32, tag="G", bufs=4, name="v_ps")
                    for jj in range(4):
                        for cci in range(CCH):
                            nc.tensor.matmul(
                                v_ps[:, jj * HD : (jj + 1) * HD],
                                lhsT=x_t[:, cci * TCH + jj * 128 : cci * TCH + (jj + 1) * 128],
                                rhs=wv_s[:, cci * HD : (cci + 1) * HD],
                                start=(cci == 0), stop=(cci == CCH - 1),
                            )
                    nc.any.tensor_copy(vnat[:, tsl], v_ps[:])

                    q_parts = []
                    for h in range(QH):
                        q_ps = psp.tile([128, TCH], F32, tag="G", bufs=4, name="q_ps")
                        for cci in range(CCH):
                            nc.tensor.matmul(
                                q_ps[:], lhsT=wq_s[:, cci * QD + h * HD : cci * QD + (h + 1) * HD],
                                rhs=x_t[:, cci * TCH : (cci + 1) * TCH],
                                start=(cci == 0), stop=(cci == CCH - 1),
                            )
                        q_parts.append(rms_sums(q_ps, is_k=False))

                    # ---- pass B: K-norm column-ify + RoPE chains ----
                    # K: srk column per key block via 4 tiny matmuls, then one copy
                    kct = psp.tile([128, 4], F32, tag="ln", bufs=2, name="kct")
                    for b in range(4):
                        nc.tensor.matmul(
                            kct[:, b : b + 1],
                            lhsT=k_rin[0:1, b * 128 : (b + 1) * 128],
                            rhs=ones_b[0:1, 0:1],
                            start=(b == 0), stop=(b == 3),
                        )
                    nc.scalar.activation(
                        srk_sb[:, tci * 4 : (tci + 1) * 4], kct[:, 0:4], AF.Copy
                    )
                    u, t2 = rope_mix(k_xs, cc_t, ss_t)
                    nc.vector.tensor_add(khat[:, tsl], u[:], t2[:])

                    # Q: broadcast 1/rms via f32r ones-column matmul, then normalize
                    for h in range(QH):
                        q_xs, q_rin = q_parts[h]
                        rbc = psp.tile([128, TCH], F32, tag="y", bufs=2, name="rbc")
                        nc.tensor.matmul(
                            rbc[:],
                            lhsT=ones_r[:],
                            rhs=q_rin[:],
                            start=True, stop=True,
                        )
                        u, t2 = rope_mix(q_xs, cc_t, ss_t)
                        nc.vector.tensor_add(u[:], u[:], t2[:])
                        nc.vector.tensor_mul(qhat[h][:, tsl], u[:], rbc[:])

                def attn(si):
                    qoff, qlen = SUBS[si]
                    kb_tot = (qoff + qlen) // 128
                    dstart = qoff // 128  # first diagonal key block
                    nsub = qlen // 128
                    LEAD = 3  # score blocks emitted ahead of their y/l pair
                    ps = {}

                    def s_exp(h, j):
                        off = max(0, (j - dstart) * 128)
                        s_ps = psp.tile([128, TCH], F32, tag="G", bufs=4, name="s_ps")
                        nc.tensor.matmul(
                            s_ps[:, off:qlen],
                            lhsT=khat[:, j * 128 : (j + 1) * 128],
                            rhs=qhat[h][:, qoff + off : qoff + qlen],
                            start=True, stop=True,
                        )
                        if j >= dstart:
                            nc.vector.tensor_add(
                                s_ps[:, off : off + 128],
                                s_ps[:, off : off + 128],
                                mask_tri[:],
                            )
                        p = pat.tile([128, TCH], BF16, tag="p", bufs=8, name="p")
                        nc.scalar.activation(
                            p[:, off:qlen], s_ps[:, off:qlen], AF.Exp,
                            scale=srk_sb[:, j : j + 1],
                        )
                        ps[(h, j)] = p

                    blocks = [(h, j) for h in range(QH) for j in range(kb_tot)]
                    for h, j in blocks[:LEAD]:
                        s_exp(h, j)
                    y_nat = l_nat = None
                    for idx, (h, j) in enumerate(blocks):
                        if idx + LEAD < len(blocks):
                            s_exp(*blocks[idx + LEAD])
                        if j == 0:
                            y_nat = psp.tile([128, TCH], F32, tag="y", bufs=2, name="y_nat")
                            l_nat = psp.tile([128, 4], F32, tag="ln", bufs=2, name="l_nat")
                        p = ps.pop((h, j))
                        # one lazy-zeroed accumulation group per PSUM tile: start
                        # only on the first write, stop only on the last; each
                        # b-block's first touch lands while its bytes are pending
                        for b in range(max(0, j - dstart), nsub):
                            nc.tensor.matmul(
                                y_nat[:, b * 128 : (b + 1) * 128],
                                lhsT=p[:, b * 128 : (b + 1) * 128],
                                rhs=vnat[:, j * 128 : (j + 1) * 128],
                                start=(j == 0 and b == 0), stop=(j == kb_tot - 1),
                            )
                            nc.tensor.matmul(
                                l_nat[:, b : b + 1],
                                lhsT=p[:, b * 128 : (b + 1) * 128],
                                rhs=ones_b[:, 0:1],
                                start=(j == 0 and b == 0), stop=(j == kb_tot - 1),
                            )
                        if j == kb_tot - 1:
                            rcl = pat.tile([128, 4], F32, tag="rcl", bufs=2, name="rcl")
                            nc.vector.reciprocal(rcl[:, :nsub], l_nat[:, :nsub])
                            yh_nat = pat.tile([128, TCH], BF16, tag="yhn", bufs=2, name="yh_nat")
                            for b in range(nsub):
                                nc.vector.tensor_scalar_mul(
                                    yh_nat[:, b * 128 : (b + 1) * 128],
                                    y_nat[:, b * 128 : (b + 1) * 128],
                                    rcl[:, b : b + 1],
                                )
                            yhT = pat.tile([128, TCH], BF16, tag="yhT", bufs=2, name="yhT")
                            if TMODE == "xbar":
                                nc.sync.dma_start(
                                    yhT[:, :qlen].rearrange("p (b t) -> p b t", t=128),
                                    yh_nat[:, :qlen],
                                    transpose=True,
                                )
                            else:
                                # out[d, t] = sum_k yh_nat[k, d] * eye[k, t]
                                yT = psp.tile([128, TCH], F32, tag="y", bufs=2, name="yT")
                                for b in range(nsub):
                                    nc.tensor.matmul(
                                        yT[:, b * 128 : (b + 1) * 128],
                                        lhsT=yh_nat[:, b * 128 : (b + 1) * 128],
                                        rhs=eye_s[:],
                                        start=(b == 0), stop=(b == nsub - 1),
                                    )
                                nc.scalar.activation(yhT[:, :qlen], yT[:, :qlen], AF.Copy)
                            nc.sync.dma_start(
                                y_loc[si][h * 128 : (h + 1) * 128, :], yhT[:, :qlen]
                            )

                def allgather(si):
                    if NO_COLLECTIVE:
                        for q in range(TP):
                            nc.sync.dma_start(
                                y_all[si][q * QD : (q + 1) * QD, :], y_loc[si][:]
                            )
                    else:
                        import concourse.mybir as mybir

                        nc.gpsimd.collective_compute(
                            "AllGather",
                            mybir.AluOpType.bypass,
                            replica_groups=[[0, 1, 2, 3], [4, 5, 6, 7]],
                            ins=[y_loc[si][:]],
                            outs=[y_all[si][:]],
                        )

                yb_tiles = {}

                def oread(si):
                    # issue the gathered-y readback as soon as the AllGather is
                    # ordered, so it never queues behind later DMA traffic
                    qoff, qlen = SUBS[si]
                    yb = pat.tile([128, CCH * TCH], BF16, tag="yb", bufs=2, name="yb")
                    src = y_all[si][:].rearrange("(cci p) t -> p cci t", p=128)
                    dst = yb[:, : CCH * qlen].rearrange("p (cci t) -> p cci t", t=qlen)
                    nc.sync.dma_start(dst, src)
                    yb_tiles[si] = yb

                def outproj(si):
                    qoff, qlen = SUBS[si]
                    yb = yb_tiles.pop(si)
                    ob = pat.tile([128, 4 * TCH], F32, tag="ob", bufs=1, name="ob")
                    for jq in range(4):
                        o_ps = psp.tile([128, TCH], F32, tag="G", bufs=4, name="o_ps")
                        for cci in range(CCH):
                            nc.tensor.matmul(
                                o_ps[:, :qlen],
                                lhsT=wo_s[:, cci * QD + jq * 128 : cci * QD + (jq + 1) * 128],
                                rhs=yb[:, cci * qlen : (cci + 1) * qlen],
                                start=(cci == 0), stop=(cci == CCH - 1),
                            )
                        nc.scalar.activation(
                            ob[:, jq * qlen : (jq + 1) * qlen], o_ps[:, :qlen], AF.Copy
                        )
                    src = ob[:, : 4 * qlen].rearrange("p (jq t) -> p jq t", jq=4)
                    dst = outT[:, qoff : qoff + qlen].rearrange("(jq p) t -> p jq t", p=128)
                    nc.sync.dma_start(dst, src)

                # ---- emission schedule ----
                proj(0)
                proj(1)
                attn(0); allgather(0); oread(0)
                proj(2)
                attn(1); allgather(1); oread(1); outproj(0)
                proj(3)
                attn(2); allgather(2); oread(2); outproj(1)
                attn(3); allgather(3); oread(3); outproj(2)
                attn(4); allgather(4); oread(4); outproj(3)
                attn(5); allgather(5); oread(5); outproj(4)
                outproj(5)

    nc.compile()
    return nc


def _get_nc():
    if "nc" not in _CACHE:
        _CACHE["nc"] = _build_nc()
    return _CACHE["nc"]


def _lay(wT):
    """[C, M] (already transposed weight) -> [128, (C/128)*M] with channel
    blocks as column groups: out[p, cci*M + j] = wT[cci*128 + p, j]."""
    Cd, M = wT.shape
    return np.ascontiguousarray(
        wT.reshape(Cd // 128, 128, M).transpose(1, 0, 2).reshape(128, -1)
    )


def _host_constants():
    if "consts" in _CACHE:
        return _CACHE["consts"]
    inv_freq = 1.0 / (ROPE_BASE ** (np.arange(0, HD, 2, dtype=np.float64) / HD))
    freqs = np.outer(np.arange(T, dtype=np.float64), inv_freq)  # [T, 64]
    cos = np.cos(freqs).astype(np.float32).T  # [64, T]
    sin = np.sin(freqs).astype(np.float32).T
    ccT = np.ascontiguousarray(np.concatenate([cos, cos], axis=0))   # [128, T]
    # the kernel computes swap(x*ss) (swap applied AFTER the multiply), so the
    # sin table is pre-swapped: swap(x)*[+sin;-sin] == swap(x*[-sin;+sin])
    ssT = np.ascontiguousarray(np.concatenate([-sin, sin], axis=0))  # [128, T]
    ii = np.arange(128, dtype=np.int64)[:, None]
    cc = np.arange(128, dtype=np.int64)[None, :]
    masks = np.where(cc >= ii, 0.0, NEG).astype(np.float32)
    _CACHE["consts"] = (ccT, ssT, masks)
    return _CACHE["consts"]


def _in_maps(x, Wq, Wk, Wv, Wo):
    import ml_dtypes

    BF = ml_dtypes.bfloat16
    ccT, ssT, masks = _host_constants()
    maps = []
    for c in range(NCORES):
        b, r = divmod(c, TP)
        xT = x[b].T.astype(BF)  # [C, T]
        x_l = np.concatenate(
            [_lay(xT[:, t * TCH : (t + 1) * TCH]) for t in range(TC)], axis=1
        )
        maps.append(
            {
                "x_in": np.ascontiguousarray(x_l),
                "wq_in": _lay(Wq[r * QD : (r + 1) * QD, :].T.astype(BF)),
                "wk_in": _lay(Wk[r * HD : (r + 1) * HD, :].T.astype(BF)),
                "wv_in": _lay(Wv[r * HD : (r + 1) * HD, :].T.astype(BF)),
                "wo_in": _lay(Wo[r * QD : (r + 1) * QD, :].T.astype(BF)),
                "cc_in": ccT,
                "ss_in": ssT,
                "mask_in": masks,
                "eye_in": np.eye(128, dtype=BF),
            }
        )
    return maps


def _assemble(results):
    out = np.empty((B, T, C), dtype=np.float32)
    for c in range(NCORES):
        b, r = divmod(c, TP)
        out[b, :, r * QD : (r + 1) * QD] = results[c]["outT"].T
    return out


def kernel(x, Wq, Wk, Wv, Wo):
    from concourse.bass_utils import run_bass_kernel_spmd

    nc = _get_nc()
    maps = _in_maps(np.asarray(x), np.asarray(Wq), np.asarray(Wk), np.asarray(Wv), np.asarray(Wo))
    res = run_bass_kernel_spmd(nc, maps, list(range(NCORES)))
    return _assemble(res.results)
